# revision 26
# baseline (speedup 1.0000x reference)
"""Trainium2 Bass kernel for nn_Detector (nms_detection post-processing).

Computes, for inputs box (N,HW,A,4), box_confidence (N,HW,A,1),
class_score (N,HW,A,C), prior (HW,A,4), feat_size:
  box_out     = decoded corners / feat_size, masked by (max score > 0.5)
  probs_out   = max_c(conf*score) masked
  class_index = argmax_c(conf*score)  (first occurrence, int32)

Sharding: pure data-parallel over batch N=32 across 8 cores (4 samples/core).

Exactness notes (vs the jax reference):
 - max_c fl(conf*s_c) == fl(conf * max_c s_c) bitwise (rounding is monotone).
 - argmax (mode "scan3", 3 DVE passes): a per-80-block running-max scan
   of RAW class scores (state=(mask*state) max cs, mask=0 at block starts)
   yields prefix maxima; the product tie-set {c: fl(conf*cs_c) == pm} is
   thresholded in the raw domain via s_min = min{s: fl(conf*s) >= pm},
   found by a 5-candidate 1-ulp window walk around fl(pm*fl(1/conf))
   (GPSIMD int32 adds do the ulp steps exactly; DVE int adds are inexact).
   count(prefix < s_min) = FIRST index attaining the product max,
   reproducing jnp.argmax tie semantics exactly (incl. conf==0 -> 0).
 - prior w/h are pre-scaled by +-0.5, and box/prior_xy by 1/feat_size, on
   the host; power-of-two scalings commute with rounding, so all box
   arithmetic matches the reference bit-for-bit.
"""

import sys
import os

for _p in ("/opt/pypackages", "/opt/trn_rl_repo"):
    if _p not in sys.path and os.path.isdir(_p):
        sys.path.append(_p)

import numpy as np

# Problem shapes (hardcoded per contract)
N, HW, A, C = 32, 4096, 5, 80
NCORES = 8
N_LOC = N // NCORES              # samples per core
APC = N_LOC * HW * A             # anchors per core (81920)
J = 32                           # anchors per partition row per tile
TILE_ANCHORS = 128 * J           # 4096
NT = APC // TILE_ANCHORS         # 20 tiles per core
S_TILES = HW * A // TILE_ANCHORS  # 5 prior tiles (prior repeats per sample)
THRESHOLD = 0.5

_CACHE = {}


def _build_program(feat_size: float, repeat: int = 1, mode: str = "scan3",
                   prod_engine: str = "vector", box_engine: str = "gpsimd",
                   small_engine: str = "vector", bufs_big: int = 2,
                   hw_loop: int = 0):
    """Build the Bacc program once; returns (nc, meta)."""
    import concourse.bacc as bacc
    import concourse.mybir as mybir
    from concourse.tile import TileContext

    dt = mybir.dt
    Alu = mybir.AluOpType
    f32, i32 = dt.float32, dt.int32
    inv_feat = 1.0 / feat_size   # exact for power-of-two feat_size

    nc = bacc.Bacc("TRN2", target_bir_lowering=False, debug=False,
                   num_devices=NCORES)

    cs = nc.dram_tensor("cs", [APC, C], f32, kind="ExternalInput")
    box = nc.dram_tensor("box", [APC, 4], f32, kind="ExternalInput")
    conf = nc.dram_tensor("conf", [APC], f32, kind="ExternalInput")
    pxy = nc.dram_tensor("pxy", [HW * A, 4], f32, kind="ExternalInput")
    pwh = nc.dram_tensor("pwh", [HW * A, 4], f32, kind="ExternalInput")
    wrev = nc.dram_tensor("wrev", [128, C], f32, kind="ExternalInput")
    kvec = nc.dram_tensor("kvec", [128, 8], i32, kind="ExternalInput")
    obox = nc.dram_tensor("obox", [APC, 4], f32, kind="ExternalOutput")
    oprb = nc.dram_tensor("oprb", [APC], f32, kind="ExternalOutput")
    oidx = nc.dram_tensor("oidx", [APC], i32, kind="ExternalOutput")

    X = mybir.AxisListType.X

    cs_v = cs[:].rearrange("(t p j) c -> t p (j c)", p=128, j=J)
    box_v = box[:].rearrange("(t p j) d -> t p (j d)", p=128, j=J)
    conf_v = conf[:].rearrange("(t p j) -> t p j", p=128, j=J)
    pxy_v = pxy[:].rearrange("(s p j) d -> s p (j d)", p=128, j=J)
    pwh_v = pwh[:].rearrange("(s p j) d -> s p (j d)", p=128, j=J)
    obox_v = obox[:].rearrange("(t p j) d -> t p (j d)", p=128, j=J)
    oprb_v = oprb[:].rearrange("(t p j) -> t p j", p=128, j=J)
    oidx_v = oidx[:].rearrange("(t p j) -> t p j", p=128, j=J)

    with TileContext(nc) as tc:
        with (
            tc.tile_pool(name="csp", bufs=3) as csp,
            tc.tile_pool(name="eqp", bufs=bufs_big) as eqp,
            tc.tile_pool(name="rwp", bufs=bufs_big) as rwp,
            tc.tile_pool(name="smp", bufs=4) as smp,
            tc.tile_pool(name="bxp", bufs=3) as bxp,
            tc.tile_pool(name="cst", bufs=1) as cst,
        ):
            # constants
            wrev_t = cst.tile([128, C], f32, tag="wrev")
            nc.sync.dma_start(out=wrev_t[:], in_=wrev[:])
            kvec_t = cst.tile([128, 8], i32, tag="kvec")
            nc.sync.dma_start(out=kvec_t[:], in_=kvec[:])
            if mode in ("scan", "scan3"):
                # block-boundary mask: 0 at c==0 of each 80-block, 1 elsewhere
                bmask_t = cst.tile([128, J * C], f32, tag="bmask")
                nc.vector.memset(bmask_t[:], 1.0)
                nc.vector.memset(
                    bmask_t[:].rearrange("p (j c) -> p j c", c=C)[:, :, 0:1], 0.0)
            pxy_ts = []
            pwh_ts = []
            for s in range(S_TILES):
                pt = cst.tile([128, J * 4], f32, tag=f"pxy{s}")
                nc.sync.dma_start(out=pt[:], in_=pxy_v[s])
                pxy_ts.append(pt)
                qt = cst.tile([128, J * 4], f32, tag=f"pwh{s}")
                nc.sync.dma_start(out=qt[:], in_=pwh_v[s])
                pwh_ts.append(qt)

            import contextlib
            loop_cm = (tc.For_i(0, hw_loop, 1) if hw_loop
                       else contextlib.nullcontext())
            with loop_cm:
              for t_rep in range(repeat * NT):
                t = t_rep % NT
                s = t % S_TILES
                cs_t = csp.tile([128, J * C], f32, tag="cs")
                nc.sync.dma_start(out=cs_t[:], in_=cs_v[t])
                cs3 = cs_t[:].rearrange("p (j c) -> p j c", c=C)

                conf_t = smp.tile([128, J], f32, tag="conf")
                nc.sync.dma_start(out=conf_t[:], in_=conf_v[t])
                box_t = bxp.tile([128, J * 4], f32, tag="box")
                nc.sync.dma_start(out=box_t[:], in_=box_v[t])

                if mode == "scan3":
                    # 3 big DVE passes: raw prefix-max scan, is_lt vs s_min,
                    # count-reduce. The product pass is replaced by an exact
                    # raw-domain threshold s_min = min{s: fl(conf*s) >= pm},
                    # found by a 2-ulp window walk around fl(pm*fl(1/conf)).
                    # Float ulp-stepping uses GPSIMD int32 adds (exact;
                    # DVE int add goes through the f32 pipe and is not).
                    scan_t = rwp.tile([128, J * C], f32, tag="scan")
                    nc.vector.tensor_tensor_scan(
                        scan_t[:], bmask_t[:], cs_t[:], 0.0,
                        op0=Alu.mult, op1=Alu.max)
                    scan3 = scan_t[:].rearrange("p (j c) -> p j c", c=C)
                    cmax = scan3[:, :, 79]          # raw per-anchor max
                    # pm = fl(conf*cmax) == reference max of products
                    pm_t = smp.tile([128, J], f32, tag="pm")
                    nc.gpsimd.tensor_mul(pm_t[:], conf_t[:], cmax)
                    pm = pm_t[:]
                    rc = smp.tile([128, J], f32, tag="rc")
                    nc.vector.reciprocal(rc[:], conf_t[:])
                    s0 = smp.tile([128, J], f32, tag="s0")
                    nc.gpsimd.tensor_mul(s0[:], pm_t[:], rc[:])
                    s0i = s0[:].bitcast(i32)
                    # candidates s0+k ulp, k=-2..2; count passing tests
                    scand = smp.tile([128, J * 5], i32, tag="scand")
                    scand3 = scand[:].rearrange("p (j k) -> p j k", k=5)
                    nc.gpsimd.tensor_tensor(
                        scand3,
                        s0i[:, :, None].broadcast_to([128, J, 5]),
                        kvec_t[:, 0:5][:, None, :].broadcast_to([128, J, 5]),
                        op=Alu.add)
                    gall = smp.tile([128, J * 5], f32, tag="gall")
                    gall3 = gall[:].rearrange("p (j k) -> p j k", k=5)
                    nc.gpsimd.tensor_mul(
                        gall3,
                        conf_t[:][:, :, None].broadcast_to([128, J, 5]),
                        scand3.bitcast(f32))
                    aall = smp.tile([128, J * 5], f32, tag="aall")
                    aall3 = aall[:].rearrange("p (j k) -> p j k", k=5)
                    nc.vector.tensor_tensor(
                        aall3, gall3,
                        pm_t[:][:, :, None].broadcast_to([128, J, 5]),
                        op=Alu.is_ge)
                    ssum = smp.tile([128, J], f32, tag="ssum")
                    nc.vector.tensor_reduce(out=ssum[:], in_=aall3, axis=X,
                                            op=Alu.add)
                    # monotone tests over k=-2..2: first passing k = 3 - count
                    toff = smp.tile([128, J], i32, tag="toff")
                    nc.vector.tensor_scalar(toff[:], ssum[:], -1.0, 3.0,
                                            op0=Alu.mult, op1=Alu.add)
                    smin = smp.tile([128, J], i32, tag="smin")
                    nc.gpsimd.tensor_tensor(smin[:], s0i, toff[:], op=Alu.add)
                    # count of (prefix < s_min) = first index with
                    # fl(conf*cs_c) == pm  (reference argmax, ties exact)
                    sminf_b = (smin[:].bitcast(f32)[:, :, None]
                               .broadcast_to([128, J, C]))
                    lt_t = eqp.tile([128, J * C], f32, tag="lt")
                    lt3 = lt_t[:].rearrange("p (j c) -> p j c", c=C)
                    nc.vector.tensor_tensor(lt3, scan3, sminf_b, op=Alu.is_lt)
                    idx_t = smp.tile([128, J], i32, tag="idx")
                    with nc.allow_low_precision("int32 count of <=80 is exact"):
                        nc.vector.tensor_reduce(out=idx_t[:], in_=lt3,
                                                axis=X, op=Alu.add)
                    nc.sync.dma_start(out=oidx_v[t], in_=idx_t[:])
                    prod3 = None
                else:
                    # ---- scores = conf * class_score (reference products) ----
                    conf_b = conf_t[:][:, :, None].broadcast_to([128, J, C])
                    prod_t = eqp.tile([128, J * C], f32, tag="prod")
                    prod3 = prod_t[:].rearrange("p (j c) -> p j c", c=C)
                    prod_eng = getattr(nc, prod_engine)
                    prod_eng.tensor_mul(prod3, cs3, conf_b)

                if mode == "scan3":
                    pass
                elif mode == "scan":
                    # running max with per-block reset:
                    #   state = (bmask*state) max prod
                    scan_t = rwp.tile([128, J * C], f32, tag="scan")
                    nc.vector.tensor_tensor_scan(
                        scan_t[:], bmask_t[:], prod_t[:], 0.0,
                        op0=Alu.mult, op1=Alu.max)
                    scan3 = scan_t[:].rearrange("p (j c) -> p j c", c=C)
                    # pm = last prefix value of each block (== block max),
                    # read in place as a strided slice
                    pm = scan3[:, :, 79]
                    # count of prefix<pm = first argmax index (exact w/ ties)
                    pm_b = scan3[:, :, 79:80].broadcast_to([128, J, C])
                    nc.vector.tensor_tensor(prod3, scan3, pm_b, op=Alu.is_lt)
                    idx_t = smp.tile([128, J], i32, tag="idx")
                    with nc.allow_low_precision("int32 count of <=80 is exact"):
                        nc.vector.tensor_reduce(out=idx_t[:], in_=prod3,
                                                axis=X, op=Alu.add)
                    nc.sync.dma_start(out=oidx_v[t], in_=idx_t[:])
                else:
                    # pm = per-anchor max of products (bitexact vs reference)
                    pm_t = smp.tile([128, J], f32, tag="pm")
                    nc.vector.tensor_reduce(out=pm_t[:], in_=prod3, axis=X,
                                            op=Alu.max)
                    pm = pm_t[:]

                    # ---- argmax: first index attaining pm ----
                    pm_b = pm_t[:][:, :, None].broadcast_to([128, J, C])
                    eq_t = rwp.tile([128, J * C], f32, tag="eq")
                    eq3 = eq_t[:].rearrange("p (j c) -> p j c", c=C)
                    nc.vector.tensor_tensor(eq3, prod3, pm_b, op=Alu.is_ge)
                    wrev_b = wrev_t[:][:, None, :].broadcast_to([128, J, C])
                    nc.vector.tensor_mul(eq3, eq3, wrev_b)  # eq*(128-c)
                    r = smp.tile([128, J], f32, tag="r")
                    nc.vector.tensor_reduce(out=r[:], in_=eq3, axis=X,
                                            op=Alu.max)

                    # idx = (128 - r) * (r > 0)  (handles conf==0 -> idx 0)
                    i1 = smp.tile([128, J], f32, tag="i1")
                    nc.vector.tensor_scalar(i1[:], r[:], -1.0, 128.0,
                                            op0=Alu.mult, op1=Alu.add)
                    mr = smp.tile([128, J], f32, tag="mr")
                    nc.vector.tensor_scalar(mr[:], r[:], 0.0, None,
                                            op0=Alu.is_gt)
                    idx_t = smp.tile([128, J], i32, tag="idx")
                    nc.vector.tensor_mul(idx_t[:], i1[:], mr[:])
                    nc.sync.dma_start(out=oidx_v[t], in_=idx_t[:])

                # ---- probs + masks ----
                sm_eng2 = getattr(nc, small_engine)
                m01 = smp.tile([128, J], f32, tag="m01")
                sm_eng2.tensor_scalar(m01[:], pm, THRESHOLD, None,
                                      op0=Alu.is_gt)
                prb = smp.tile([128, J], f32, tag="prb")
                sm_eng2.tensor_mul(prb[:], pm, m01[:])
                nc.sync.dma_start(out=oprb_v[t], in_=prb[:])

                # ---- box decode ----
                box3 = box_t[:].rearrange("p (j d) -> p j d", d=4)
                bxy2 = box3[:, :, None, 0:2].broadcast_to([128, J, 2, 2])
                bwh2 = box3[:, :, None, 2:4].broadcast_to([128, J, 2, 2])
                pxy3 = pxy_ts[s][:].rearrange("p (j k d) -> p j k d", k=2, d=2)
                pwh3 = pwh_ts[s][:].rearrange("p (j k d) -> p j k d", k=2, d=2)
                box_eng = getattr(nc, box_engine)
                t1 = bxp.tile([128, J * 4], f32, tag="t1")
                t1v = t1[:].rearrange("p (j k d) -> p j k d", k=2, d=2)
                box_eng.tensor_mul(t1v, bwh2, pwh3)       # -+half
                a1 = bxp.tile([128, J * 4], f32, tag="a1")
                a1v = a1[:].rearrange("p (j k d) -> p j k d", k=2, d=2)
                box_eng.tensor_add(a1v, bxy2, pxy3)       # xy + prior_xy
                cr = bxp.tile([128, J * 4], f32, tag="cr")
                crv = cr[:].rearrange("p (j k d) -> p j k d", k=2, d=2)
                box_eng.tensor_add(crv, a1v, t1v)         # corners
                ob = bxp.tile([128, J * 4], f32, tag="ob")
                obv = ob[:].rearrange("p (j k d) -> p j k d", k=2, d=2)
                m01_b = m01[:][:, :, None, None].broadcast_to([128, J, 2, 2])
                box_eng.tensor_mul(obv, crv, m01_b)       # mask (inputs pre-scaled)
                nc.sync.dma_start(out=obox_v[t], in_=ob[:])

    nc.finalize()
    return nc


def _prep_host(box, box_confidence, class_score, prior, feat_size):
    inv = np.float32(1.0 / float(feat_size))   # exact for power-of-two
    box = np.ascontiguousarray(np.asarray(box, dtype=np.float32)
                               .reshape(N * HW * A, 4) * inv)
    conf = np.ascontiguousarray(np.asarray(box_confidence, dtype=np.float32)
                                .reshape(N * HW * A))
    cs = np.ascontiguousarray(np.asarray(class_score, dtype=np.float32)
                              .reshape(N * HW * A, C))
    prior = np.asarray(prior, dtype=np.float32).reshape(HW * A, 4)
    pxy = np.ascontiguousarray(
        np.concatenate([prior[:, 0:2], prior[:, 0:2]], axis=1) * inv)
    half = prior[:, 2:4] * np.float32(0.5)
    pwh = np.ascontiguousarray(np.concatenate([-half, half], axis=1))
    wrev = np.ascontiguousarray(
        np.broadcast_to((128.0 - np.arange(C, dtype=np.float32))[None, :],
                        (128, C))).astype(np.float32)
    kv = np.zeros((128, 8), np.int32)
    kv[:, 0:5] = np.arange(-2, 3, dtype=np.int32)[None, :]
    return box, conf, cs, pxy, pwh, wrev, kv


def kernel(box, box_confidence, class_score, prior, feat_size):
    from concourse.bass_utils import run_bass_kernel_spmd

    fs = float(feat_size)
    key = ("prog", fs)
    if key not in _CACHE:
        _CACHE[key] = _build_program(fs)
    nc = _CACHE[key]

    boxf, conf, cs, pxy, pwh, wrev, kv = _prep_host(
        box, box_confidence, class_score, prior, feat_size)

    in_maps = []
    for c in range(NCORES):
        lo, hi = c * APC, (c + 1) * APC
        in_maps.append({
            "cs": cs[lo:hi],
            "box": boxf[lo:hi],
            "conf": conf[lo:hi],
            "pxy": pxy,
            "pwh": pwh,
            "wrev": wrev,
            "kvec": kv,
        })

    res = run_bass_kernel_spmd(nc, in_maps, core_ids=list(range(NCORES)))

    box_out = np.concatenate([r["obox"] for r in res.results]) \
        .reshape(N, HW, A, 4)
    probs_out = np.concatenate([r["oprb"] for r in res.results]) \
        .reshape(N, HW, A)
    class_index = np.concatenate([r["oidx"] for r in res.results]) \
        .reshape(N, HW, A).astype(np.int32)
    return box_out, probs_out, class_index


if __name__ == "__main__":
    import reference as R
    inp = R.setup_inputs()
    out = kernel(**{k: np.asarray(v) if not np.isscalar(v) else v
                    for k, v in inp.items()})
    print([o.shape for o in out], [o.dtype for o in out])


# revision 27
# speedup vs baseline: 1.8233x; 1.8233x over previous
"""Trainium2 Bass kernel for nn_Detector (nms_detection post-processing).

Computes, for inputs box (N,HW,A,4), box_confidence (N,HW,A,1),
class_score (N,HW,A,C), prior (HW,A,4), feat_size:
  box_out     = decoded corners / feat_size, masked by (max score > 0.5)
  probs_out   = max_c(conf*score) masked
  class_index = argmax_c(conf*score)  (first occurrence, int32)

Sharding: pure data-parallel over batch N=32 across 8 cores (4 samples/core).

Exactness notes (vs the jax reference):
 - max_c fl(conf*s_c) == fl(conf * max_c s_c) bitwise (rounding is monotone).
 - argmax (mode "scan3", 3 DVE passes): a per-80-block running-max scan
   of RAW class scores (state=(mask*state) max cs, mask=0 at block starts)
   yields prefix maxima; the product tie-set {c: fl(conf*cs_c) == pm} is
   thresholded in the raw domain via s_min = min{s: fl(conf*s) >= pm},
   found by a 5-candidate 1-ulp window walk around fl(pm*fl(1/conf))
   (GPSIMD int32 adds do the ulp steps exactly; DVE int adds are inexact).
   count(prefix < s_min) = FIRST index attaining the product max,
   reproducing jnp.argmax tie semantics exactly (incl. conf==0 -> 0).
 - prior w/h are pre-scaled by +-0.5, and box/prior_xy by 1/feat_size, on
   the host; power-of-two scalings commute with rounding, so all box
   arithmetic matches the reference bit-for-bit.
"""

import sys
import os

for _p in ("/opt/pypackages", "/opt/trn_rl_repo"):
    if _p not in sys.path and os.path.isdir(_p):
        sys.path.append(_p)

import numpy as np

# Problem shapes (hardcoded per contract)
N, HW, A, C = 32, 4096, 5, 80
NCORES = 8
N_LOC = N // NCORES              # samples per core
APC = N_LOC * HW * A             # anchors per core (81920)
J = 32                           # anchors per partition row per tile
TILE_ANCHORS = 128 * J           # 4096
NT = APC // TILE_ANCHORS         # 20 tiles per core
S_TILES = HW * A // TILE_ANCHORS  # 5 prior tiles (prior repeats per sample)
THRESHOLD = 0.5

_CACHE = {}


def _build_program(feat_size: float, repeat: int = 1, mode: str = "scan3",
                   prod_engine: str = "vector", box_engine: str = "gpsimd",
                   small_engine: str = "vector", bufs_big: int = 2,
                   hw_loop: int = 0):
    """Build the Bacc program once; returns (nc, meta)."""
    import concourse.bacc as bacc
    import concourse.mybir as mybir
    from concourse.tile import TileContext

    dt = mybir.dt
    Alu = mybir.AluOpType
    f32, i32 = dt.float32, dt.int32
    inv_feat = 1.0 / feat_size   # exact for power-of-two feat_size

    nc = bacc.Bacc("TRN2", target_bir_lowering=False, debug=False,
                   num_devices=NCORES)

    cs = nc.dram_tensor("cs", [APC, C], f32, kind="ExternalInput")
    box = nc.dram_tensor("box", [APC, 4], f32, kind="ExternalInput")
    conf = nc.dram_tensor("conf", [APC], f32, kind="ExternalInput")
    pxy = nc.dram_tensor("pxy", [HW * A, 4], f32, kind="ExternalInput")
    pwh = nc.dram_tensor("pwh", [HW * A, 4], f32, kind="ExternalInput")
    wrev = nc.dram_tensor("wrev", [128, C], f32, kind="ExternalInput")
    kvec = nc.dram_tensor("kvec", [128, 8], i32, kind="ExternalInput")
    obox = nc.dram_tensor("obox", [APC, 4], f32, kind="ExternalOutput")
    oprb = nc.dram_tensor("oprb", [APC], f32, kind="ExternalOutput")
    oidx = nc.dram_tensor("oidx", [APC], i32, kind="ExternalOutput")

    X = mybir.AxisListType.X

    cs_v = cs[:].rearrange("(t p j) c -> t p (j c)", p=128, j=J)
    box_v = box[:].rearrange("(t p j) d -> t p (j d)", p=128, j=J)
    conf_v = conf[:].rearrange("(t p j) -> t p j", p=128, j=J)
    pxy_v = pxy[:].rearrange("(s p j) d -> s p (j d)", p=128, j=J)
    pwh_v = pwh[:].rearrange("(s p j) d -> s p (j d)", p=128, j=J)
    obox_v = obox[:].rearrange("(t p j) d -> t p (j d)", p=128, j=J)
    oprb_v = oprb[:].rearrange("(t p j) -> t p j", p=128, j=J)
    oidx_v = oidx[:].rearrange("(t p j) -> t p j", p=128, j=J)

    with TileContext(nc) as tc:
        with (
            tc.tile_pool(name="csp", bufs=4) as csp,
            tc.tile_pool(name="eqp", bufs=bufs_big) as eqp,
            tc.tile_pool(name="rwp", bufs=bufs_big + 1) as rwp,
            tc.tile_pool(name="smp", bufs=8) as smp,
            tc.tile_pool(name="bxp", bufs=4) as bxp,
            tc.tile_pool(name="cst", bufs=1) as cst,
        ):
            # constants
            wrev_t = cst.tile([128, C], f32, tag="wrev")
            nc.sync.dma_start(out=wrev_t[:], in_=wrev[:])
            kvec_t = cst.tile([128, 8], i32, tag="kvec")
            nc.sync.dma_start(out=kvec_t[:], in_=kvec[:])
            if mode in ("scan", "scan3"):
                # block-boundary mask: 0 at c==0 of each 80-block, 1 elsewhere
                bmask_t = cst.tile([128, J * C], f32, tag="bmask")
                nc.vector.memset(bmask_t[:], 1.0)
                nc.vector.memset(
                    bmask_t[:].rearrange("p (j c) -> p j c", c=C)[:, :, 0:1], 0.0)
            pxy_ts = []
            pwh_ts = []
            for s in range(S_TILES):
                pt = cst.tile([128, J * 4], f32, tag=f"pxy{s}")
                nc.sync.dma_start(out=pt[:], in_=pxy_v[s])
                pxy_ts.append(pt)
                qt = cst.tile([128, J * 4], f32, tag=f"pwh{s}")
                nc.sync.dma_start(out=qt[:], in_=pwh_v[s])
                pwh_ts.append(qt)

            import contextlib
            loop_cm = (tc.For_i(0, hw_loop, 1) if hw_loop
                       else contextlib.nullcontext())
            with loop_cm:
              for t_rep in range(repeat * NT):
                t = t_rep % NT
                s = t % S_TILES
                cs_t = csp.tile([128, J * C], f32, tag="cs")
                nc.sync.dma_start(out=cs_t[:], in_=cs_v[t])
                cs3 = cs_t[:].rearrange("p (j c) -> p j c", c=C)

                conf_t = smp.tile([128, J], f32, tag="conf")
                nc.sync.dma_start(out=conf_t[:], in_=conf_v[t])
                box_t = bxp.tile([128, J * 4], f32, tag="box")
                nc.sync.dma_start(out=box_t[:], in_=box_v[t])

                if mode == "scan3":
                    # 3 big DVE passes: raw prefix-max scan, is_lt vs s_min,
                    # count-reduce. The product pass is replaced by an exact
                    # raw-domain threshold s_min = min{s: fl(conf*s) >= pm},
                    # found by a 2-ulp window walk around fl(pm*fl(1/conf)).
                    # Float ulp-stepping uses GPSIMD int32 adds (exact;
                    # DVE int add goes through the f32 pipe and is not).
                    scan_t = rwp.tile([128, J * C], f32, tag="scan")
                    nc.vector.tensor_tensor_scan(
                        scan_t[:], bmask_t[:], cs_t[:], 0.0,
                        op0=Alu.mult, op1=Alu.max)
                    scan3 = scan_t[:].rearrange("p (j c) -> p j c", c=C)
                    cmax = scan3[:, :, 79]          # raw per-anchor max
                    # pm = fl(conf*cmax) == reference max of products
                    pm_t = smp.tile([128, J], f32, tag="pm")
                    nc.gpsimd.tensor_mul(pm_t[:], conf_t[:], cmax)
                    pm = pm_t[:]
                    rc = smp.tile([128, J], f32, tag="rc")
                    nc.vector.reciprocal(rc[:], conf_t[:])
                    s0 = smp.tile([128, J], f32, tag="s0")
                    nc.gpsimd.tensor_mul(s0[:], pm_t[:], rc[:])
                    s0i = s0[:].bitcast(i32)
                    # candidates s0+k ulp, k=-2..2; count passing tests
                    scand = smp.tile([128, J * 5], i32, tag="scand")
                    scand3 = scand[:].rearrange("p (j k) -> p j k", k=5)
                    nc.gpsimd.tensor_tensor(
                        scand3,
                        s0i[:, :, None].broadcast_to([128, J, 5]),
                        kvec_t[:, 0:5][:, None, :].broadcast_to([128, J, 5]),
                        op=Alu.add)
                    gall = smp.tile([128, J * 5], f32, tag="gall")
                    gall3 = gall[:].rearrange("p (j k) -> p j k", k=5)
                    nc.gpsimd.tensor_mul(
                        gall3,
                        conf_t[:][:, :, None].broadcast_to([128, J, 5]),
                        scand3.bitcast(f32))
                    aall = smp.tile([128, J * 5], f32, tag="aall")
                    aall3 = aall[:].rearrange("p (j k) -> p j k", k=5)
                    nc.vector.tensor_tensor(
                        aall3, gall3,
                        pm_t[:][:, :, None].broadcast_to([128, J, 5]),
                        op=Alu.is_ge)
                    ssum = smp.tile([128, J], f32, tag="ssum")
                    nc.vector.tensor_reduce(out=ssum[:], in_=aall3, axis=X,
                                            op=Alu.add)
                    # monotone tests over k=-2..2: first passing k = 3 - count
                    toff = smp.tile([128, J], i32, tag="toff")
                    nc.vector.tensor_scalar(toff[:], ssum[:], -1.0, 3.0,
                                            op0=Alu.mult, op1=Alu.add)
                    smin = smp.tile([128, J], i32, tag="smin")
                    nc.gpsimd.tensor_tensor(smin[:], s0i, toff[:], op=Alu.add)
                    # count of (prefix < s_min) = first index with
                    # fl(conf*cs_c) == pm  (reference argmax, ties exact)
                    sminf_b = (smin[:].bitcast(f32)[:, :, None]
                               .broadcast_to([128, J, C]))
                    lt_t = eqp.tile([128, J * C], f32, tag="lt")
                    lt3 = lt_t[:].rearrange("p (j c) -> p j c", c=C)
                    nc.vector.tensor_tensor(lt3, scan3, sminf_b, op=Alu.is_lt)
                    idx_t = smp.tile([128, J], i32, tag="idx")
                    with nc.allow_low_precision("int32 count of <=80 is exact"):
                        nc.vector.tensor_reduce(out=idx_t[:], in_=lt3,
                                                axis=X, op=Alu.add)
                    nc.sync.dma_start(out=oidx_v[t], in_=idx_t[:])
                    prod3 = None
                else:
                    # ---- scores = conf * class_score (reference products) ----
                    conf_b = conf_t[:][:, :, None].broadcast_to([128, J, C])
                    prod_t = eqp.tile([128, J * C], f32, tag="prod")
                    prod3 = prod_t[:].rearrange("p (j c) -> p j c", c=C)
                    prod_eng = getattr(nc, prod_engine)
                    prod_eng.tensor_mul(prod3, cs3, conf_b)

                if mode == "scan3":
                    pass
                elif mode == "scan":
                    # running max with per-block reset:
                    #   state = (bmask*state) max prod
                    scan_t = rwp.tile([128, J * C], f32, tag="scan")
                    nc.vector.tensor_tensor_scan(
                        scan_t[:], bmask_t[:], prod_t[:], 0.0,
                        op0=Alu.mult, op1=Alu.max)
                    scan3 = scan_t[:].rearrange("p (j c) -> p j c", c=C)
                    # pm = last prefix value of each block (== block max),
                    # read in place as a strided slice
                    pm = scan3[:, :, 79]
                    # count of prefix<pm = first argmax index (exact w/ ties)
                    pm_b = scan3[:, :, 79:80].broadcast_to([128, J, C])
                    nc.vector.tensor_tensor(prod3, scan3, pm_b, op=Alu.is_lt)
                    idx_t = smp.tile([128, J], i32, tag="idx")
                    with nc.allow_low_precision("int32 count of <=80 is exact"):
                        nc.vector.tensor_reduce(out=idx_t[:], in_=prod3,
                                                axis=X, op=Alu.add)
                    nc.sync.dma_start(out=oidx_v[t], in_=idx_t[:])
                else:
                    # pm = per-anchor max of products (bitexact vs reference)
                    pm_t = smp.tile([128, J], f32, tag="pm")
                    nc.vector.tensor_reduce(out=pm_t[:], in_=prod3, axis=X,
                                            op=Alu.max)
                    pm = pm_t[:]

                    # ---- argmax: first index attaining pm ----
                    pm_b = pm_t[:][:, :, None].broadcast_to([128, J, C])
                    eq_t = rwp.tile([128, J * C], f32, tag="eq")
                    eq3 = eq_t[:].rearrange("p (j c) -> p j c", c=C)
                    nc.vector.tensor_tensor(eq3, prod3, pm_b, op=Alu.is_ge)
                    wrev_b = wrev_t[:][:, None, :].broadcast_to([128, J, C])
                    nc.vector.tensor_mul(eq3, eq3, wrev_b)  # eq*(128-c)
                    r = smp.tile([128, J], f32, tag="r")
                    nc.vector.tensor_reduce(out=r[:], in_=eq3, axis=X,
                                            op=Alu.max)

                    # idx = (128 - r) * (r > 0)  (handles conf==0 -> idx 0)
                    i1 = smp.tile([128, J], f32, tag="i1")
                    nc.vector.tensor_scalar(i1[:], r[:], -1.0, 128.0,
                                            op0=Alu.mult, op1=Alu.add)
                    mr = smp.tile([128, J], f32, tag="mr")
                    nc.vector.tensor_scalar(mr[:], r[:], 0.0, None,
                                            op0=Alu.is_gt)
                    idx_t = smp.tile([128, J], i32, tag="idx")
                    nc.vector.tensor_mul(idx_t[:], i1[:], mr[:])
                    nc.sync.dma_start(out=oidx_v[t], in_=idx_t[:])

                # ---- probs + masks ----
                sm_eng2 = getattr(nc, small_engine)
                m01 = smp.tile([128, J], f32, tag="m01")
                sm_eng2.tensor_scalar(m01[:], pm, THRESHOLD, None,
                                      op0=Alu.is_gt)
                prb = smp.tile([128, J], f32, tag="prb")
                nc.gpsimd.tensor_mul(prb[:], pm, m01[:])
                nc.sync.dma_start(out=oprb_v[t], in_=prb[:])

                # ---- box decode ----
                box3 = box_t[:].rearrange("p (j d) -> p j d", d=4)
                bxy2 = box3[:, :, None, 0:2].broadcast_to([128, J, 2, 2])
                bwh2 = box3[:, :, None, 2:4].broadcast_to([128, J, 2, 2])
                pxy3 = pxy_ts[s][:].rearrange("p (j k d) -> p j k d", k=2, d=2)
                pwh3 = pwh_ts[s][:].rearrange("p (j k d) -> p j k d", k=2, d=2)
                box_eng = getattr(nc, box_engine)
                t1 = bxp.tile([128, J * 4], f32, tag="t1")
                t1v = t1[:].rearrange("p (j k d) -> p j k d", k=2, d=2)
                box_eng.tensor_mul(t1v, bwh2, pwh3)       # -+half
                a1 = bxp.tile([128, J * 4], f32, tag="a1")
                a1v = a1[:].rearrange("p (j k d) -> p j k d", k=2, d=2)
                box_eng.tensor_add(a1v, bxy2, pxy3)       # xy + prior_xy
                cr = bxp.tile([128, J * 4], f32, tag="cr")
                crv = cr[:].rearrange("p (j k d) -> p j k d", k=2, d=2)
                box_eng.tensor_add(crv, a1v, t1v)         # corners
                ob = bxp.tile([128, J * 4], f32, tag="ob")
                obv = ob[:].rearrange("p (j k d) -> p j k d", k=2, d=2)
                m01_b = m01[:][:, :, None, None].broadcast_to([128, J, 2, 2])
                box_eng.tensor_mul(obv, crv, m01_b)       # mask (inputs pre-scaled)
                nc.sync.dma_start(out=obox_v[t], in_=ob[:])

    nc.finalize()
    return nc


def _prep_host(box, box_confidence, class_score, prior, feat_size):
    inv = np.float32(1.0 / float(feat_size))   # exact for power-of-two
    box = np.ascontiguousarray(np.asarray(box, dtype=np.float32)
                               .reshape(N * HW * A, 4) * inv)
    conf = np.ascontiguousarray(np.asarray(box_confidence, dtype=np.float32)
                                .reshape(N * HW * A))
    cs = np.ascontiguousarray(np.asarray(class_score, dtype=np.float32)
                              .reshape(N * HW * A, C))
    prior = np.asarray(prior, dtype=np.float32).reshape(HW * A, 4)
    pxy = np.ascontiguousarray(
        np.concatenate([prior[:, 0:2], prior[:, 0:2]], axis=1) * inv)
    half = prior[:, 2:4] * np.float32(0.5)
    pwh = np.ascontiguousarray(np.concatenate([-half, half], axis=1))
    wrev = np.ascontiguousarray(
        np.broadcast_to((128.0 - np.arange(C, dtype=np.float32))[None, :],
                        (128, C))).astype(np.float32)
    kv = np.zeros((128, 8), np.int32)
    kv[:, 0:5] = np.arange(-2, 3, dtype=np.int32)[None, :]
    return box, conf, cs, pxy, pwh, wrev, kv


def kernel(box, box_confidence, class_score, prior, feat_size):
    from concourse.bass_utils import run_bass_kernel_spmd

    fs = float(feat_size)
    key = ("prog", fs)
    if key not in _CACHE:
        _CACHE[key] = _build_program(fs)
    nc = _CACHE[key]

    boxf, conf, cs, pxy, pwh, wrev, kv = _prep_host(
        box, box_confidence, class_score, prior, feat_size)

    in_maps = []
    for c in range(NCORES):
        lo, hi = c * APC, (c + 1) * APC
        in_maps.append({
            "cs": cs[lo:hi],
            "box": boxf[lo:hi],
            "conf": conf[lo:hi],
            "pxy": pxy,
            "pwh": pwh,
            "wrev": wrev,
            "kvec": kv,
        })

    res = run_bass_kernel_spmd(nc, in_maps, core_ids=list(range(NCORES)))

    box_out = np.concatenate([r["obox"] for r in res.results]) \
        .reshape(N, HW, A, 4)
    probs_out = np.concatenate([r["oprb"] for r in res.results]) \
        .reshape(N, HW, A)
    class_index = np.concatenate([r["oidx"] for r in res.results]) \
        .reshape(N, HW, A).astype(np.int32)
    return box_out, probs_out, class_index


if __name__ == "__main__":
    import reference as R
    inp = R.setup_inputs()
    out = kernel(**{k: np.asarray(v) if not np.isscalar(v) else v
                    for k, v in inp.items()})
    print([o.shape for o in out], [o.dtype for o in out])
